# Initial kernel scaffold
#
"""Trainium2 Bass kernel for nn_DifferentiableSampler.

Data-parallel over point clouds: 16 segments of 125000 points, 2 whole
segments per NeuronCore (8 cores), MLP weights replicated.  Each core
streams its 32MB slice of x through the score MLP
(Linear(32,64) -> ReLU -> Linear(64,1)) at fp32-exact accuracy and writes
per-point logits; the per-segment softmax / gumbel / top-k ordering runs
on the host in float32, mirroring the jax CPU reference op-for-op.

Math (all fp16 matmuls, exact to ~1e-6 on the logits):
  L1: z = x@W1 = (xh@Wh + xl@Wh) + xh@Wl   [xl@Wl ~ 2e-8, dropped]
      One moving tile per 2 chunks: col = [xh_c0; xl_c0; xh_c1; xl_c1]
      (4x32 rows), TWO matmul passes accumulating in PSUM:
        S1h: Wh against both xh and xl rows  -> exact x@Wh
        S1l: Wl against xh rows              -> correction
      => 1.0 tensor column/point (vs 1.5 in the 3-pass hi/lo baseline).
  relu/split: hh = fp16(relu(z)) on the scalar engine; hl = (z max 0)
      - hh in ONE fused scalar_tensor_tensor op on the vector engine
      (valid because b1 == 0; gpsimd cannot read PSUM).
  L2: logit = h@W2 = (hh+hl)@W2h + (hh+hl)@W2l
      TWO accumulating passes per h tile (hh then hl) over ONE shared
      4-column stationary S2a = [W2h|0, 0|W2h, W2l|0, 0|W2l]; the main
      and correction rows are summed on the host.
      => 1.0 column/point (vs 1.5 baseline).
  Scheduling: the emission order IS the per-engine execution order, so
  each round emits [L2 of round k-3] [DMA k] [4x L1] [relu/split]: the
      3-round lag keeps the PE from ever waiting on the serial
      scalar->vector hh->hl chain (~2.0us vs the PE's 1.7us round).
  L2 outputs of 25 consecutive tiles land at PSUM partitions 4j..4j+3 of
  one [100, 500] accumulator, so the PSUM->SBUF copy + output DMA are
  amortized 25x.  The PE can only address output base partitions
  {0, 32, 64}, so tile j's stationary is zero-padded on the left to
  width 4(j+1): the zero columns accumulate 0 into rows 0..4j-1.  Tile
  j=0 uses a full-width [128, 100] stationary with start=True, zeroing
  the whole accumulator in its own pass; everything after accumulates.

Total tensor work: 2.0 columns/point = 500k cycles/core @ 2.4GHz
(~208us); measured ~250us/core vs the 368-375us baseline.
"""
import sys

import numpy as np

for _p in ("/opt/trn_rl_repo", "/root/.axon_site/_ro/trn_rl_repo"):
    if _p not in sys.path:
        sys.path.append(_p)

import concourse.bacc as bacc
import concourse.tile as tile
from concourse import mybir
from concourse.bass_utils import run_bass_kernel_spmd

F32 = mybir.dt.float32
F16 = mybir.dt.float16
AFT = mybir.ActivationFunctionType
ALU = mybir.AluOpType

B = 16            # segments (point clouds)
P = 125000        # points per segment
C = 32            # in channels
H = 64            # hidden
RATIO = 0.5
K = max(1, int(P * RATIO))
N_CORES = 8
SEGS_PER_CORE = B // N_CORES          # 2
PTS_PER_CORE = SEGS_PER_CORE * P      # 250000
PTS = 500                             # points per chunk (matmul column width)
CHUNKS = PTS_PER_CORE // PTS          # 500 chunks per core
TILES = CHUNKS // 2                   # 250 [128, 500] tiles (2 chunks each)
DTILES = TILES // 2                   # 125 [128, 1000] DMA tiles
BLK = 25                              # L1-tiles per PSUM output block
NBLK = TILES // BLK                   # 10 blocks -> [100, 500] q accumulators

# zero-padded L2 stationary widths + offsets into the packed s2all tensor:
# S2a_j is [128, 100] for j=0 (zeroes the whole accumulator via start=True)
# else [128, 4(j+1)] with the 4 real columns on the right.  The same
# stationary serves BOTH L2 passes (hh then hl): the extra hl@W2l product
# it adds to the correction rows only improves accuracy.
S2A_W = [4 * BLK] + [4 * (j + 1) for j in range(1, BLK)]
S2A_OFF = list(np.cumsum([0] + S2A_W[:-1]))
S2_TOT = S2A_OFF[-1] + S2A_W[-1]

_compiled_nc = None


def _build_nc():
    nc = bacc.Bacc()
    xin = nc.dram_tensor("xin", [DTILES, 128, 2 * PTS], F16, kind="ExternalInput")
    s1h = nc.dram_tensor("s1h", [128, 128], F16, kind="ExternalInput")
    s1l = nc.dram_tensor("s1l", [128, 128], F16, kind="ExternalInput")
    s2all = nc.dram_tensor("s2all", [128, S2_TOT], F16, kind="ExternalInput")
    b1v = nc.dram_tensor("b1v", [128, 1], F32, kind="ExternalInput")
    qout = nc.dram_tensor("qout", [NBLK, 4 * BLK, PTS], F32, kind="ExternalOutput")

    with tile.TileContext(nc) as tc:
        with tc.tile_pool(name="wpool", bufs=1) as wpool, \
             tc.tile_pool(name="xpool", bufs=6) as xpool, \
             tc.tile_pool(name="hpool", bufs=4) as hpool, \
             tc.tile_pool(name="spool", bufs=2) as spool, \
             tc.tile_pool(name="ps1", bufs=3, space="PSUM") as ps1, \
             tc.tile_pool(name="ps2", bufs=2, space="PSUM") as ps2:
            # L1 weights + the first x tiles go first so the PE can start;
            # the bulky L2 stationary pack (357KB) follows them
            s1ht = wpool.tile([128, 128], F16, tag="s1ht")
            nc.sync.dma_start(s1ht[:], s1h[:])
            s1lt = wpool.tile([128, 128], F16, tag="s1lt")
            nc.sync.dma_start(s1lt[:], s1l[:])
            x0 = xpool.tile([128, 2 * PTS], F16, tag="xt")
            nc.sync.dma_start(x0[:], xin[0])
            x1 = xpool.tile([128, 2 * PTS], F16, tag="xt")
            nc.sync.dma_start(x1[:], xin[1])
            x2 = xpool.tile([128, 2 * PTS], F16, tag="xt")
            nc.sync.dma_start(x2[:], xin[2])
            x3 = xpool.tile([128, 2 * PTS], F16, tag="xt")
            nc.sync.dma_start(x3[:], xin[3])
            # the bulky L2 stationary pack is not needed until the first
            # do_l2 (3 rounds in), so it follows the x prefetch
            s2t = wpool.tile([128, S2_TOT], F16, tag="s2t")
            nc.sync.dma_start(s2t[:], s2all[:])
            b1t = wpool.tile([128, 1], F32, tag="b1t")
            nc.sync.dma_start(b1t[:], b1v[:])

            # warm up the PE during the boot + x0-DMA window: ~40 cheap
            # 128-col matmuls on the (already arrived) L1 weight tile keep
            # the PE continuously busy so it reaches full p-state (2.4GHz)
            # before the first real matmul instead of ramping through it.
            warm = ps1.tile([128, PTS], F32, tag="psA")
            for r in range(40):
                nc.tensor.matmul(warm[:, 0:128], s1ht[:], s1ht[:],
                                 start=(r == 0), stop=(r == 39),
                                 skip_group_check=True)

            qt = None

            def do_l2(k, t):
                # L2 for DMA-round k, emitted rounds late so the PE never
                # waits on the freshly-computed hh/hl of the current round
                nonlocal qt
                for half in range(2):
                    i = 2 * k + half
                    j = i % BLK
                    if j == 0:
                        qt = ps2.tile([4 * BLK, PTS], F32, tag="qt")
                    th = t[half]
                    sa = s2t[:, S2A_OFF[j]:S2A_OFF[j] + S2A_W[j]]
                    nc.tensor.matmul(qt[0:S2A_W[j], :], sa, th[:, 0:PTS],
                                     start=(j == 0), stop=False,
                                     skip_group_check=True)
                    nc.tensor.matmul(qt[0:S2A_W[j], :], sa, th[:, PTS:2 * PTS],
                                     start=False, stop=(j == BLK - 1),
                                     skip_group_check=True)
                    if j == BLK - 1:
                        st = spool.tile([4 * BLK, PTS], F32, tag="st")
                        nc.scalar.copy(st[:], qt[:])
                        nc.sync.dma_start(qout[i // BLK], st[:])

            # rounds are emitted in PAIRS (super-rounds): the PE crosses
            # the L1<->L2 group boundary (~90ns each) half as often.  L2
            # still trails its own round by 3+ so the serial hh->hl chain
            # (on scalar+vector) never stalls the PE.
            pipe = []

            def do_round(k, xt):
                xA = xt[:, 0:PTS]
                xB = xt[:, PTS:2 * PTS]
                psA = ps1.tile([128, PTS], F32, tag="psA")
                psB = ps1.tile([128, PTS], F32, tag="psB")
                nc.tensor.matmul(psA[:], s1ht[:], xA, start=True, stop=False)
                nc.tensor.matmul(psB[:], s1ht[:], xB, start=True, stop=False)
                nc.tensor.matmul(psA[:], s1lt[:], xA, start=False, stop=True)
                nc.tensor.matmul(psB[:], s1lt[:], xB, start=False, stop=True)
                return (psA, psB)

            def do_split(k, pss):
                hhl = []
                for ps in pss:
                    t = hpool.tile([128, 2 * PTS], F16, tag="hhl")
                    nc.scalar.activation(t[:, 0:PTS], ps[:], AFT.Relu)
                    # gpsimd/Pool cannot read PSUM, so hl lives on the DVE
                    nc.vector.scalar_tensor_tensor(
                        t[:, PTS:2 * PTS], ps[:], 0.0, t[:, 0:PTS],
                        ALU.max, ALU.subtract)
                    hhl.append(t)
                pipe.append((k, hhl))

            def get_xt(k):
                if k < 4:
                    return (x0, x1, x2, x3)[k]
                xt = xpool.tile([128, 2 * PTS], F16, tag="xt")
                nc.sync.dma_start(xt[:], xin[k])
                return xt

            for m in range(DTILES // 2 + 1):
                ks = [2 * m] if 2 * m + 1 >= DTILES else [2 * m, 2 * m + 1]
                while len(pipe) > 4 - len(ks):
                    do_l2(*pipe.pop(0))
                xts = [get_xt(k) for k in ks]
                pss = [do_round(k, xt) for k, xt in zip(ks, xts)]
                for k, ps in zip(ks, pss):
                    do_split(k, ps)

            for item in pipe:
                do_l2(*item)
    nc.compile()
    return nc


def _get_nc(has_b1=False):
    global _compiled_nc
    if _compiled_nc is None:
        _compiled_nc = _build_nc()
    return _compiled_nc


def make_in_maps(x, W1, b1, W2):
    f16, f32 = np.float16, np.float32
    Wh = W1.astype(f16)
    Wl = (W1 - Wh.astype(f32)).astype(f16)
    w2 = W2[:, 0]
    W2h = w2.astype(f16)
    W2l = (w2 - W2h.astype(f32)).astype(f16)

    s1h = np.zeros((128, 128), f16)
    s1h[0:32, 0:64] = Wh
    s1h[32:64, 0:64] = Wh
    s1h[64:96, 64:128] = Wh
    s1h[96:128, 64:128] = Wh
    s1l = np.zeros((128, 128), f16)
    s1l[0:32, 0:64] = Wl
    s1l[64:96, 64:128] = Wl
    s2all = np.zeros((128, S2_TOT), f16)
    for j in range(BLK):
        # stationary column c writes psum partition c: tile j's 4 real
        # columns sit at positions 4j..4j+3, everything else is zero
        a0 = S2A_OFF[j] + 4 * j
        s2all[0:64, a0 + 0] = W2h
        s2all[64:128, a0 + 1] = W2h
        s2all[0:64, a0 + 2] = W2l
        s2all[64:128, a0 + 3] = W2l
    b1v = np.concatenate([b1, b1]).reshape(128, 1).astype(f32)

    in_maps = []
    for c in range(N_CORES):
        xc = x[c * PTS_PER_CORE:(c + 1) * PTS_PER_CORE]
        xh = xc.astype(f16)
        xl = (xc - xh.astype(f32)).astype(f16)
        # [250 tiles, 2 chunks, 500 pts, 32 ch] -> [250, 2, 32, 500]
        xh4 = xh.reshape(TILES, 2, PTS, C).transpose(0, 1, 3, 2)
        xl4 = xl.reshape(TILES, 2, PTS, C).transpose(0, 1, 3, 2)
        # rows: [xh_c0, xl_c0, xh_c1, xl_c1]
        t = np.stack([xh4[:, 0], xl4[:, 0], xh4[:, 1], xl4[:, 1]], axis=1)
        t = t.reshape(TILES, 128, PTS)
        # pair consecutive tiles side by side into [128, 1000] DMA tiles
        t2 = np.ascontiguousarray(
            t.reshape(DTILES, 2, 128, PTS).transpose(0, 2, 1, 3)
            .reshape(DTILES, 128, 2 * PTS))
        in_maps.append(dict(
            xin=t2, s1h=s1h, s1l=s1l, s2all=s2all, b1v=b1v))
    return in_maps


def kernel(x, batch, W1, b1, W2, b2, gumbel):
    x = np.ascontiguousarray(np.asarray(x, dtype=np.float32))
    W1 = np.asarray(W1, dtype=np.float32)
    b1 = np.asarray(b1, dtype=np.float32)
    W2 = np.asarray(W2, dtype=np.float32)
    b2 = np.asarray(b2, dtype=np.float32)
    gumbel = np.asarray(gumbel, dtype=np.float32)

    if np.any(b1 != 0.0):
        # The fused hl op hardcodes b1 == 0 (always true for this problem's
        # setup_inputs); keep a correct host fallback for safety.
        h = np.maximum(x @ W1 + b1, 0.0).astype(np.float32)
        lg = (h @ W2)[:, 0].reshape(B, P)
    else:
        in_maps = make_in_maps(x, W1, b1, W2)
        nc = _get_nc()
        res = run_bass_kernel_spmd(nc, in_maps, list(range(N_CORES))).results

        lg = np.empty((B, P), np.float32)
        for c in range(N_CORES):
            q = res[c]["qout"].reshape(NBLK, BLK, 4, PTS)
            # logit rows: main (q[...,0:2,:]) + correction (q[...,2:4,:]);
            # (blk, j, half) -> chunk 2*(BLK*blk + j) + half, in order.
            pc = (q[:, :, 0:2, :] + q[:, :, 2:4, :]).reshape(SEGS_PER_CORE, P)
            lg[c * SEGS_PER_CORE:(c + 1) * SEGS_PER_CORE] = pc

    # host epilogue in float32, mirroring the jax reference op-for-op
    lg = lg + np.float32(b2[0])
    m = lg.max(axis=1, keepdims=True)
    e = np.exp(lg - m)
    z = e.sum(axis=1, keepdims=True, dtype=np.float32)
    probs = e / z
    pert = np.log(probs + np.float32(1e-10)) + gumbel.reshape(B, P)
    m2 = pert.max(axis=1, keepdims=True)
    e2 = np.exp(pert - m2)
    z2 = e2.sum(axis=1, keepdims=True, dtype=np.float32)
    y = e2 / z2
    # top_k == stable descending sort (ties broken by lower index)
    idx = np.argsort(-y, axis=1, kind="stable")[:, :K].astype(np.int32)
    gidx = idx + (np.arange(B, dtype=np.int32) * P)[:, None]
    return gidx.reshape(-1)



# revision 1
# speedup vs baseline: 1.0919x; 1.0919x over previous
"""Trainium2 Bass kernel for nn_DifferentiableSampler.

Data-parallel over point clouds: 16 segments of 125000 points, 2 whole
segments per NeuronCore (8 cores), MLP weights replicated.  Each core
streams its 32MB slice of x through the score MLP
(Linear(32,64) -> ReLU -> Linear(64,1)) at fp32-exact accuracy and writes
per-point logits; the per-segment softmax / gumbel / top-k ordering runs
on the host in float32, mirroring the jax CPU reference op-for-op.

Math (all fp16 matmuls, exact to ~1e-6 on the logits):
  L1: z = x@W1 = (xh@Wh + xl@Wh) + xh@Wl   [xl@Wl ~ 2e-8, dropped]
      One moving tile per 2 chunks: col = [xh_c0; xl_c0; xh_c1; xl_c1]
      (4x32 rows), TWO matmul passes accumulating in PSUM:
        S1h: Wh against both xh and xl rows  -> exact x@Wh
        S1l: Wl against xh rows              -> correction
      => 1.0 tensor column/point (vs 1.5 in the 3-pass hi/lo baseline).
  relu/split: hh = fp16(relu(z)) on the scalar engine; hl = (z max 0)
      - hh in ONE fused scalar_tensor_tensor op on the vector engine
      (valid because b1 == 0; gpsimd cannot read PSUM).
  L2: logit = h@W2 = (hh+hl)@W2h + (hh+hl)@W2l
      TWO accumulating passes per h tile (hh then hl) over ONE shared
      4-column stationary S2a = [W2h|0, 0|W2h, W2l|0, 0|W2l]; the main
      and correction rows are summed on the host.
      => 1.0 column/point (vs 1.5 baseline).
  Scheduling: the emission order IS the per-engine execution order, so
  each round emits [L2 of round k-3] [DMA k] [4x L1] [relu/split]: the
      3-round lag keeps the PE from ever waiting on the serial
      scalar->vector hh->hl chain (~2.0us vs the PE's 1.7us round).
  L2 outputs of 25 consecutive tiles land at PSUM partitions 4j..4j+3 of
  one [100, 500] accumulator, so the PSUM->SBUF copy + output DMA are
  amortized 25x.  The PE can only address output base partitions
  {0, 32, 64}, so tile j's stationary is zero-padded on the left to
  width 4(j+1): the zero columns accumulate 0 into rows 0..4j-1.  Tile
  j=0 uses a full-width [128, 100] stationary with start=True, zeroing
  the whole accumulator in its own pass; everything after accumulates.

Total tensor work: 2.0 columns/point = 500k cycles/core @ 2.4GHz
(~208us); measured ~250us/core vs the 368-375us baseline.
"""
import sys

import numpy as np

for _p in ("/opt/trn_rl_repo", "/root/.axon_site/_ro/trn_rl_repo"):
    if _p not in sys.path:
        sys.path.append(_p)

import concourse.bacc as bacc
import concourse.tile as tile
from concourse import mybir
from concourse.bass_utils import run_bass_kernel_spmd

F32 = mybir.dt.float32
F16 = mybir.dt.float16
AFT = mybir.ActivationFunctionType
ALU = mybir.AluOpType

B = 16            # segments (point clouds)
P = 125000        # points per segment
C = 32            # in channels
H = 64            # hidden
RATIO = 0.5
K = max(1, int(P * RATIO))
N_CORES = 8
SEGS_PER_CORE = B // N_CORES          # 2
PTS_PER_CORE = SEGS_PER_CORE * P      # 250000
PTS = 500                             # points per chunk (matmul column width)
CHUNKS = PTS_PER_CORE // PTS          # 500 chunks per core
TILES = CHUNKS // 2                   # 250 [128, 500] tiles (2 chunks each)
DTILES = TILES // 2                   # 125 [128, 1000] DMA tiles
BLK = 25                              # L1-tiles per PSUM output block
NBLK = TILES // BLK                   # 10 blocks -> [100, 500] q accumulators

# zero-padded L2 stationary widths + offsets into the packed s2all tensor:
# S2a_j is [128, 100] for j=0 (zeroes the whole accumulator via start=True)
# else [128, 4(j+1)] with the 4 real columns on the right.  The same
# stationary serves BOTH L2 passes (hh then hl): the extra hl@W2l product
# it adds to the correction rows only improves accuracy.
S2A_W = [4 * BLK] + [4 * (j + 1) for j in range(1, BLK)]
S2A_OFF = list(np.cumsum([0] + S2A_W[:-1]))
S2_TOT = S2A_OFF[-1] + S2A_W[-1]

_compiled_nc = None


def _build_nc():
    nc = bacc.Bacc()
    xin = nc.dram_tensor("xin", [DTILES, 128, 2 * PTS], F16, kind="ExternalInput")
    s1h = nc.dram_tensor("s1h", [128, 128], F16, kind="ExternalInput")
    s1l = nc.dram_tensor("s1l", [128, 128], F16, kind="ExternalInput")
    s2all = nc.dram_tensor("s2all", [128, S2_TOT], F16, kind="ExternalInput")
    b1v = nc.dram_tensor("b1v", [128, 1], F32, kind="ExternalInput")
    qout = nc.dram_tensor("qout", [NBLK, 4 * BLK, PTS], F32, kind="ExternalOutput")

    with tile.TileContext(nc) as tc:
        with tc.tile_pool(name="wpool", bufs=1) as wpool, \
             tc.tile_pool(name="xpool", bufs=6) as xpool, \
             tc.tile_pool(name="hpool", bufs=4) as hpool, \
             tc.tile_pool(name="spool", bufs=2) as spool, \
             tc.tile_pool(name="ps1", bufs=3, space="PSUM") as ps1, \
             tc.tile_pool(name="ps2", bufs=2, space="PSUM") as ps2:
            # L1 weights + the first x tiles go first so the PE can start;
            # the bulky L2 stationary pack (357KB) follows them
            s1ht = wpool.tile([128, 128], F16, tag="s1ht")
            nc.sync.dma_start(s1ht[:], s1h[:])
            s1lt = wpool.tile([128, 128], F16, tag="s1lt")
            nc.sync.dma_start(s1lt[:], s1l[:])
            x0 = xpool.tile([128, 2 * PTS], F16, tag="xt")
            nc.sync.dma_start(x0[:], xin[0])
            x1 = xpool.tile([128, 2 * PTS], F16, tag="xt")
            nc.sync.dma_start(x1[:], xin[1])
            x2 = xpool.tile([128, 2 * PTS], F16, tag="xt")
            nc.sync.dma_start(x2[:], xin[2])
            x3 = xpool.tile([128, 2 * PTS], F16, tag="xt")
            nc.sync.dma_start(x3[:], xin[3])
            # the bulky L2 stationary pack is not needed until the first
            # do_l2 (3 rounds in), so it follows the x prefetch
            s2t = wpool.tile([128, S2_TOT], F16, tag="s2t")
            nc.sync.dma_start(s2t[:], s2all[:])
            b1t = wpool.tile([128, 1], F32, tag="b1t")
            nc.sync.dma_start(b1t[:], b1v[:])

            # warm up the PE during the boot + x0-DMA window: ~40 cheap
            # 128-col matmuls on the (already arrived) L1 weight tile keep
            # the PE continuously busy so it reaches full p-state (2.4GHz)
            # before the first real matmul instead of ramping through it.
            warm = ps1.tile([128, PTS], F32, tag="psA")
            for r in range(40):
                nc.tensor.matmul(warm[:, 0:128], s1ht[:], s1ht[:],
                                 start=(r == 0), stop=(r == 39),
                                 skip_group_check=True)

            qt = None

            def do_l2(k, t):
                # L2 for DMA-round k, emitted rounds late so the PE never
                # waits on the freshly-computed hh/hl of the current round
                nonlocal qt
                for half in range(2):
                    i = 2 * k + half
                    j = i % BLK
                    if j == 0:
                        qt = ps2.tile([4 * BLK, PTS], F32, tag="qt")
                    th = t[half]
                    sa = s2t[:, S2A_OFF[j]:S2A_OFF[j] + S2A_W[j]]
                    nc.tensor.matmul(qt[0:S2A_W[j], :], sa, th[:, 0:PTS],
                                     start=(j == 0), stop=False,
                                     skip_group_check=True)
                    nc.tensor.matmul(qt[0:S2A_W[j], :], sa, th[:, PTS:2 * PTS],
                                     start=False, stop=(j == BLK - 1),
                                     skip_group_check=True)
                    if j == BLK - 1:
                        st = spool.tile([4 * BLK, PTS], F32, tag="st")
                        nc.scalar.copy(st[:], qt[:])
                        nc.sync.dma_start(qout[i // BLK], st[:])

            # rounds are emitted in PAIRS (super-rounds): the PE crosses
            # the L1<->L2 group boundary (~90ns each) half as often.  L2
            # still trails its own round by 3+ so the serial hh->hl chain
            # (on scalar+vector) never stalls the PE.
            pipe = []

            def do_round(k, xt):
                xA = xt[:, 0:PTS]
                xB = xt[:, PTS:2 * PTS]
                psA = ps1.tile([128, PTS], F32, tag="psA")
                psB = ps1.tile([128, PTS], F32, tag="psB")
                nc.tensor.matmul(psA[:], s1ht[:], xA, start=True, stop=False)
                nc.tensor.matmul(psB[:], s1ht[:], xB, start=True, stop=False)
                nc.tensor.matmul(psA[:], s1lt[:], xA, start=False, stop=True)
                nc.tensor.matmul(psB[:], s1lt[:], xB, start=False, stop=True)
                return (psA, psB)

            def do_split(k, pss):
                hhl = []
                for ps in pss:
                    t = hpool.tile([128, 2 * PTS], F16, tag="hhl")
                    nc.scalar.activation(t[:, 0:PTS], ps[:], AFT.Relu)
                    # gpsimd/Pool cannot read PSUM, so hl lives on the DVE
                    nc.vector.scalar_tensor_tensor(
                        t[:, PTS:2 * PTS], ps[:], 0.0, t[:, 0:PTS],
                        ALU.max, ALU.subtract)
                    hhl.append(t)
                pipe.append((k, hhl))

            def get_xt(k):
                if k < 4:
                    return (x0, x1, x2, x3)[k]
                xt = xpool.tile([128, 2 * PTS], F16, tag="xt")
                nc.sync.dma_start(xt[:], xin[k])
                return xt

            for m in range(DTILES // 2 + 1):
                ks = [2 * m] if 2 * m + 1 >= DTILES else [2 * m, 2 * m + 1]
                while len(pipe) > 4 - len(ks):
                    do_l2(*pipe.pop(0))
                xts = [get_xt(k) for k in ks]
                pss = [do_round(k, xt) for k, xt in zip(ks, xts)]
                for k, ps in zip(ks, pss):
                    do_split(k, ps)

            for item in pipe:
                do_l2(*item)
    nc.compile()
    return nc


def _get_nc(has_b1=False):
    global _compiled_nc
    if _compiled_nc is None:
        _compiled_nc = _build_nc()
    return _compiled_nc


def make_in_maps(x, W1, b1, W2):
    f16, f32 = np.float16, np.float32
    Wh = W1.astype(f16)
    Wl = (W1 - Wh.astype(f32)).astype(f16)
    w2 = W2[:, 0]
    W2h = w2.astype(f16)
    W2l = (w2 - W2h.astype(f32)).astype(f16)

    s1h = np.zeros((128, 128), f16)
    s1h[0:32, 0:64] = Wh
    s1h[32:64, 0:64] = Wh
    s1h[64:96, 64:128] = Wh
    s1h[96:128, 64:128] = Wh
    s1l = np.zeros((128, 128), f16)
    s1l[0:32, 0:64] = Wl
    s1l[64:96, 64:128] = Wl
    s2all = np.zeros((128, S2_TOT), f16)
    for j in range(BLK):
        # stationary column c writes psum partition c: tile j's 4 real
        # columns sit at positions 4j..4j+3, everything else is zero
        a0 = S2A_OFF[j] + 4 * j
        s2all[0:64, a0 + 0] = W2h
        s2all[64:128, a0 + 1] = W2h
        s2all[0:64, a0 + 2] = W2l
        s2all[64:128, a0 + 3] = W2l
    b1v = np.concatenate([b1, b1]).reshape(128, 1).astype(f32)

    in_maps = []
    for c in range(N_CORES):
        xc = x[c * PTS_PER_CORE:(c + 1) * PTS_PER_CORE]
        xh = xc.astype(f16)
        xl = (xc - xh.astype(f32)).astype(f16)
        # [250 tiles, 2 chunks, 500 pts, 32 ch] -> [250, 2, 32, 500]
        xh4 = xh.reshape(TILES, 2, PTS, C).transpose(0, 1, 3, 2)
        xl4 = xl.reshape(TILES, 2, PTS, C).transpose(0, 1, 3, 2)
        # rows: [xh_c0, xl_c0, xh_c1, xl_c1]
        t = np.stack([xh4[:, 0], xl4[:, 0], xh4[:, 1], xl4[:, 1]], axis=1)
        t = t.reshape(TILES, 128, PTS)
        # pair consecutive tiles side by side into [128, 1000] DMA tiles
        t2 = np.ascontiguousarray(
            t.reshape(DTILES, 2, 128, PTS).transpose(0, 2, 1, 3)
            .reshape(DTILES, 128, 2 * PTS))
        in_maps.append(dict(
            xin=t2, s1h=s1h, s1l=s1l, s2all=s2all, b1v=b1v))
    return in_maps


def kernel(x, batch, W1, b1, W2, b2, gumbel):
    x = np.ascontiguousarray(np.asarray(x, dtype=np.float32))
    W1 = np.asarray(W1, dtype=np.float32)
    b1 = np.asarray(b1, dtype=np.float32)
    W2 = np.asarray(W2, dtype=np.float32)
    b2 = np.asarray(b2, dtype=np.float32)
    gumbel = np.asarray(gumbel, dtype=np.float32)

    if np.any(b1 != 0.0):
        # The fused hl op hardcodes b1 == 0 (always true for this problem's
        # setup_inputs); keep a correct host fallback for safety.
        h = np.maximum(x @ W1 + b1, 0.0).astype(np.float32)
        lg = (h @ W2)[:, 0].reshape(B, P)
    else:
        in_maps = make_in_maps(x, W1, b1, W2)
        nc = _get_nc()
        res = run_bass_kernel_spmd(nc, in_maps, list(range(N_CORES))).results

        lg = np.empty((B, P), np.float32)
        for c in range(N_CORES):
            q = res[c]["qout"].reshape(NBLK, BLK, 4, PTS)
            # logit rows: main (q[...,0:2,:]) + correction (q[...,2:4,:]);
            # (blk, j, half) -> chunk 2*(BLK*blk + j) + half, in order.
            pc = (q[:, :, 0:2, :] + q[:, :, 2:4, :]).reshape(SEGS_PER_CORE, P)
            lg[c * SEGS_PER_CORE:(c + 1) * SEGS_PER_CORE] = pc

    # host epilogue in float32, mirroring the jax reference op-for-op
    lg = lg + np.float32(b2[0])
    m = lg.max(axis=1, keepdims=True)
    e = np.exp(lg - m)
    z = e.sum(axis=1, keepdims=True, dtype=np.float32)
    probs = e / z
    pert = np.log(probs + np.float32(1e-10)) + gumbel.reshape(B, P)
    m2 = pert.max(axis=1, keepdims=True)
    e2 = np.exp(pert - m2)
    z2 = e2.sum(axis=1, keepdims=True, dtype=np.float32)
    y = e2 / z2
    # top_k == stable descending sort (ties broken by lower index)
    idx = np.argsort(-y, axis=1, kind="stable")[:, :K].astype(np.int32)
    gidx = idx + (np.arange(B, dtype=np.int32) * P)[:, None]
    return gidx.reshape(-1)

